# revision 30
# baseline (speedup 1.0000x reference)
"""Multi-head dot-product attention (B=2, S=2048, F=1024, H=16, DH=64, O=1024)
as a Bass/Tile kernel on 8 Trainium2 NeuronCores.

Sharding: data-parallel over B (2) x tensor-parallel over H (4 groups of 4
heads) = 8 cores. Each core computes q/k/v projections for its 4 heads,
softmax attention, and a partial output projection; the host sums the 4
partial outputs per batch element and adds the bias.

Device layouts (per core) are pre-tiled on the host so every DMA piece is
contiguous per SBUF partition (big descriptors; an f-strided source would
shatter into 1KB descriptors and cap each transfer at ~24GB/s):
  xq, xkv  [P, nch, nf, CH] fp16   x[c, ft, q] per partition
  wq, wk   [P, 2, nf, 128]  fp16   per-pair-major (wq pre-scaled 1/sqrt(DH))
  wv       [P, nf, hd]      fp16
  wo       [P, hd//P, O]    fp16
  out      [S, O]           fp16   partial output (host accumulates in f32)

Attention works in transposed-score space: sT[k, q] = KT_slice.T @ QT (two
heads packed into PE row-groups 0-63 / 64-127 run concurrently), one exp on
ACT covers both heads, then y'T = V'.T @ PT where V' carries a ones column
so row 64 of y'T accumulates the softmax denominator (scores are O(1), so
max-subtraction is unnecessary). The denominator row is partition-broadcast
on GPSIMD, reciprocated on DVE, and multiplied into fp16 yT tiles used as
lhsT of the output projection.

Scheduling, driven by two facts: (a) the ACT exp stream (~137us busy) is
the critical resource, (b) a score matmul for exp #n cannot start until
exp #n-2 frees its PSUM slot, so any PE work emitted after it inherits an
ACT-locked stall. Hence: blocks 0-2's score/exp groups dribble out during
the projection chains (filling ACT from ~14us), block 3's are emitted
through block 0's slots, and blocks 4-7 emit their own scores just-in-time
four k-slots ahead inside their own loops (first four at the previous
block's tail) so almost no PE work queues behind an ACT-locked matmul. The
hB y-stream is staggered two k-slots behind hA so its single-buffered PSUM
bank clears the previous block's inline normalization. The output
projection is split per pair: pair-0 partials (one matmul + fp16 cast)
drain during the pair-1 block, pair-1 add-units during the next block, so
only the last chunk's 8 add-units, the final normalization, and 4 stores
trail the last exp. Dummy warmup matmuls fill the chunk-0 DMA window to
hold the HAM activity up so projection chains run at 2.4GHz.
"""

import numpy as np

import concourse.bass as bass
import concourse.mybir as mybir
import concourse.tile as tile
from concourse import bacc
from concourse.bass_utils import run_bass_kernel_spmd

F32 = mybir.dt.float32
F16 = mybir.dt.float16
AF = mybir.ActivationFunctionType

B, S, F, H, DH, O = 2, 2048, 1024, 16, 64, 1024
NCORES = 8
HPC = 4  # heads per core
CH = 512  # q-chunk width
P = 128
NPAR = 3  # pt parity depth (blocks of exp tiles in flight)
JIT_LEAD = 4  # in-block score emission lead (k-slots)


def build_program(s=S, f=F, o=O, hpc=HPC):
    npair = hpc // 2
    nch = s // CH  # q chunks
    nkt = s // P  # k tiles
    nf = f // P  # contraction tiles for projections
    hd = hpc * DH  # stacked head dims per core (256)

    nc = bacc.Bacc("TRN2", target_bir_lowering=False, debug=False, num_devices=NCORES)

    xq_d = nc.dram_tensor("xq", [P, nch, nf, CH], F16, kind="ExternalInput")
    xkv_d = nc.dram_tensor("xkv", [P, nch, nf, CH], F16, kind="ExternalInput")
    wq_d = nc.dram_tensor("wq", [P, npair, nf, P], F16, kind="ExternalInput")
    wk_d = nc.dram_tensor("wk", [P, npair, nf, P], F16, kind="ExternalInput")
    wv_d = nc.dram_tensor("wv", [P, nf, hd], F16, kind="ExternalInput")
    wo_d = nc.dram_tensor("wo", [P, hd // P, o], F16, kind="ExternalInput")
    out = nc.dram_tensor("out", [s, o], F16, kind="ExternalOutput")

    with tile.TileContext(nc) as tc:
        with (
            tc.tile_pool(name="weights", bufs=1) as wpool,
            tc.tile_pool(name="xin", bufs=2) as xpool,
            tc.tile_pool(name="qkv", bufs=1) as qkvpool,
            tc.tile_pool(name="pt", bufs=1) as ptpool,
            tc.tile_pool(name="norm", bufs=2) as npool,
            tc.tile_pool(name="acc", bufs=1) as apool,
            tc.tile_pool(name="outsb", bufs=2) as opool,
        ):
            # ---- weights + constants -------------------------------------
            wq_sb = [wpool.tile([P, nf, P], F16, tag=f"wq{m}", name=f"wq{m}") for m in range(npair)]
            wk_sb = [wpool.tile([P, nf, P], F16, tag=f"wk{m}", name=f"wk{m}") for m in range(npair)]
            wv_sb = wpool.tile([P, nf, hd], F16, tag="wv")
            wo_sb = wpool.tile([P, hd // P, o], F16, tag="wo")
            # memset can't write fp16; memset fp32 scratch, cast-copy
            ones_f32 = wpool.tile([P, 4 * P], F32, tag="ones_f32")
            nc.vector.memset(ones_f32[:], 1.0)
            dummy16 = wpool.tile([1, 4 * P], F16, tag="dummy16")
            nc.vector.tensor_copy(dummy16[:], ones_f32[0:1, :])

            # ---- storage -------------------------------------------------
            QT = [
                [qkvpool.tile([P, CH], F16, tag=f"QT{p_}_{c}", name=f"QT{p_}_{c}") for c in range(nch)]
                for p_ in range(npair)
            ]
            KT = [
                [qkvpool.tile([P, CH], F16, tag=f"KT{p_}_{c}", name=f"KT{p_}_{c}") for c in range(nch)]
                for p_ in range(npair)
            ]
            # V': per k-tile [128, hpc, DH+1]; last column is ones
            V = [qkvpool.tile([P, hpc, DH + 1], F16, tag=f"V{kt}", name=f"V{kt}") for kt in range(nkt)]
            YT = [
                [qkvpool.tile([P, CH], F16, tag=f"YT{p_}_{c}", name=f"YT{p_}_{c}") for c in range(nch)]
                for p_ in range(npair)
            ]

            # ps_s (scores PSUM) lives for the whole kernel: 4 banks.
            # Projection-phase pools add 4 more (within the 8-bank budget);
            # after they close, psY0(2)+psY1(1)+ps_o(1) take the other 4.
            with tc.tile_pool(name="ps_att", bufs=2, space="PSUM") as ps_att:
                blocks = [(c, p_) for c in range(nch) for p_ in range(npair)]

                def emit_scores(p_, c, kt):
                    ps_s = ps_att.tile([P, 2 * CH], F32, tag="ps_s", name="ps_s")
                    nc.tensor.matmul(
                        ps_s[:, 0:CH],
                        KT[p_][kt // 4][0:DH, (kt % 4) * P : (kt % 4 + 1) * P],
                        QT[p_][c][0:DH, :],
                        tile_position=(0, 0),
                    )
                    nc.tensor.matmul(
                        ps_s[:, CH : 2 * CH],
                        KT[p_][kt // 4][DH : 2 * DH, (kt % 4) * P : (kt % 4 + 1) * P],
                        QT[p_][c][DH : 2 * DH, :],
                        tile_position=(DH, 0),
                    )
                    return ps_s

                # saved exp(scores) tiles; (block % NPAR, kt) keys the slot
                PT = {}

                def emit_score_exp(bi, kt):
                    c, p_ = blocks[bi]
                    ps_s = emit_scores(p_, c, kt)
                    pt = ptpool.tile(
                        [P, 2 * CH], F16, tag=f"pt{bi % NPAR}_{kt}", name=f"pt{bi % NPAR}_{kt}"
                    )
                    nc.scalar.activation(pt[:], ps_s[:], AF.Exp)
                    PT[(bi, kt)] = pt

                # prefetch bookkeeping for blocks 0..NPAR-1 (proj phase)
                pf_done = [set() for _ in range(NPAR)]
                qt_ready = [[False] * nch for _ in range(npair)]
                kt_ready = [[False] * nch for _ in range(npair)]

                def pf_drain(limit):
                    n = 0
                    for bi in range(min(NPAR, len(blocks))):
                        cb, pb = blocks[bi]
                        if not qt_ready[pb][cb]:
                            continue
                        for kt in range(nkt):
                            if n >= limit:
                                return
                            if kt in pf_done[bi] or not kt_ready[pb][kt // 4]:
                                continue
                            emit_score_exp(bi, kt)
                            pf_done[bi].add(kt)
                            n += 1

                # ---- projections (+ blocks 0-2 scores/exp woven in) ------
                with (
                    tc.tile_pool(name="ps_projqk", bufs=1, space="PSUM") as ps_projqk,
                    tc.tile_pool(name="ps_projv", bufs=2, space="PSUM") as ps_projv,
                ):
                    # PE warm-up: the HAM raises the PE clock only after
                    # ~10us of SUSTAINED activity, so fill the chunk-0 DMA
                    # wait with back-to-back dummy matmuls
                    for wu in range(12):
                        ps_wu = ps_projv.tile([P, CH], F32, tag="psV", name="ps_wu")
                        nc.tensor.matmul(ps_wu[:], dummy16[0:1, 0:P], dummy16[0:1, :])
                    for kt in range(nkt):
                        nc.vector.tensor_copy(V[kt][:, :, DH], ones_f32[:, 0:hpc])

                    XQP = [(0, 2), (2, 4), (4, 8)]  # xq piece ft ranges
                    XKP = [(0, 2), (2, 5), (5, 8)]  # xkv piece ft ranges
                    for c in range(nch):
                        cs = slice(c * CH, (c + 1) * CH)
                        xq_p = [
                            xpool.tile([P, b - a, CH], F16, tag=f"xq{i}", name=f"xq{i}")
                            for i, (a, b) in enumerate(XQP)
                        ]
                        xkv_p = [
                            xpool.tile([P, b - a, CH], F16, tag=f"xkv{i}", name=f"xkv{i}")
                            for i, (a, b) in enumerate(XKP)
                        ]

                        def xq_ap(ft):
                            for i, (a, b) in enumerate(XQP):
                                if a <= ft < b:
                                    return xq_p[i][:, ft - a]
                            raise AssertionError

                        def xkv_ap(ft):
                            for i, (a, b) in enumerate(XKP):
                                if a <= ft < b:
                                    return xkv_p[i][:, ft - a]
                            raise AssertionError

                        if c == 0:
                            # critical-path transfers on both rings, in
                            # consumption order
                            nc.sync.dma_start(wq_sb[0][:], wq_d.ap()[:, 0])
                            for i, (a, b) in enumerate(XQP):
                                nc.sync.dma_start(xq_p[i][:], xq_d.ap()[:, c, a:b, :])
                            nc.sync.dma_start(wq_sb[1][:], wq_d.ap()[:, 1])
                            nc.scalar.dma_start(wk_sb[0][:], wk_d.ap()[:, 0])
                            for i, (a, b) in enumerate(XKP):
                                nc.scalar.dma_start(xkv_p[i][:], xkv_d.ap()[:, c, a:b, :])
                            nc.scalar.dma_start(wk_sb[1][:], wk_d.ap()[:, 1])
                            nc.scalar.dma_start(wv_sb[:], wv_d.ap())
                        else:
                            if c == 1:
                                nc.sync.dma_start(wo_sb[:], wo_d.ap())
                            for i, (a, b) in enumerate(XQP):
                                nc.sync.dma_start(xq_p[i][:], xq_d.ap()[:, c, a:b, :])
                            for i, (a, b) in enumerate(XKP):
                                nc.sync.dma_start(xkv_p[i][:], xkv_d.ap()[:, c, a:b, :])

                        # per-pair projection chains; K copies land right
                        # after each pair's Q chain so its score groups can
                        # start the prefetch early
                        for m in range(npair):
                            psQ = ps_projqk.tile([P, CH], F32, tag=f"psQK{m}", name="psQ")
                            for ft in range(nf):
                                nc.tensor.matmul(
                                    psQ[:],
                                    wq_sb[m][:, ft],
                                    xq_ap(ft),
                                    start=(ft == 0),
                                    stop=(ft == nf - 1),
                                )
                            nc.vector.tensor_copy(QT[m][c][:], psQ[:])
                            qt_ready[m][c] = True
                            pf_drain(2)
                            psK = ps_projqk.tile([P, CH], F32, tag=f"psQK{m}", name="psK")
                            for ft in range(nf):
                                nc.tensor.matmul(
                                    psK[:],
                                    wk_sb[m][:, ft],
                                    xkv_ap(ft),
                                    start=(ft == 0),
                                    stop=(ft == nf - 1),
                                )
                            nc.vector.tensor_copy(KT[m][c][:], psK[:])
                            kt_ready[m][c] = True
                            pf_drain(2)
                        # V pass (xkv chunk tile as lhsT); one PSUM
                        # accumulation group per bank, so st is outer
                        for st in range(4):
                            psV = ps_projv.tile([P, CH], F32, tag="psV", name="psV")
                            for ft in range(nf):
                                nc.tensor.matmul(
                                    psV[:, 0:hd],
                                    xkv_ap(ft)[:, st * P : (st + 1) * P],
                                    wv_sb[:, ft, :],
                                    start=(ft == 0),
                                    stop=(ft == nf - 1),
                                )
                            kt = c * 4 + st
                            nc.vector.tensor_copy(
                                V[kt][:, :, 0:DH],
                                psV[:, 0:hd].rearrange("p (h d) -> p h d", h=hpc),
                            )
                            pf_drain(3)
                    pf_drain(99)

                # deferred output-projection queue: sub-microsecond PE units
                # injected into later k-slots so the PE never bursts
                pending = []

                def normalize(p_, c, psY, h01):
                    den_r = npool.tile([1, CH], F32, tag="den", name="den_r")
                    nc.vector.tensor_copy(den_r[:], psY[h01][DH : DH + 1, :])
                    bc_sb = npool.tile([DH, CH], F32, tag="bc", name="bc_sb")
                    nc.gpsimd.partition_broadcast(bc_sb[:], den_r[:])
                    inv_sb = npool.tile([DH, CH], F32, tag="inv", name="inv_sb")
                    nc.vector.reciprocal_approx_fast(out=inv_sb[:], in_=bc_sb[:])
                    nc.vector.tensor_tensor(
                        YT[p_][c][h01 * DH : (h01 + 1) * DH, :],
                        psY[h01][0:DH, :],
                        inv_sb[:],
                        mybir.AluOpType.mult,
                    )

                acc32 = {}

                def queue_outproj_m0(c):
                    # pair-0 partials into SBUF fp16 while pair 1 is still
                    # accumulating its y
                    for st in range(4):
                        for j in range(2):

                            def emit_m0(st=st, j=j, c=c):
                                ps_o = ps_opool.tile([P, CH], F32, tag="ps_o", name="ps_o")
                                nc.tensor.matmul(
                                    ps_o[:],
                                    YT[0][c][:, st * P : (st + 1) * P],
                                    wo_sb[:, 0, j * CH : (j + 1) * CH],
                                )
                                a32 = apool.tile(
                                    [P, CH], F16, tag=f"a32_{st}_{j}", name="a32"
                                )
                                nc.vector.tensor_copy(a32[:], ps_o[:])
                                acc32[(st, j)] = a32

                            pending.append(emit_m0)

                def queue_outproj_m1(c, on_psy0=False):
                    # one matmul + one fused add-cast per (st, j); the last
                    # chunk's run on the freed psY0 slots (bufs=2) so the
                    # tail chain pipelines
                    for st in range(4):
                        qt = c * 4 + st
                        carrier = {}

                        def emit_m1(j, st=st, c=c, qt=qt, carrier=carrier):
                            if j == 0:
                                carrier["out16"] = opool.tile([P, o], F16, tag="out16", name="out16")
                            if on_psy0:
                                ps_o = ps_y0pool.tile([P, CH], F32, tag="psY0", name="ps_o1")
                            else:
                                ps_o = ps_opool.tile([P, CH], F32, tag="ps_o", name="ps_o1")
                            nc.tensor.matmul(
                                ps_o[:],
                                YT[1][c][:, st * P : (st + 1) * P],
                                wo_sb[:, 1, j * CH : (j + 1) * CH],
                            )
                            nc.vector.tensor_tensor(
                                carrier["out16"][:, j * CH : (j + 1) * CH],
                                acc32.pop((st, j))[:],
                                ps_o[:],
                                mybir.AluOpType.add,
                            )
                            if j == 1:
                                nc.sync.dma_start(
                                    out.ap()[qt * P : (qt + 1) * P, :], carrier["out16"][:]
                                )

                        pending.append(lambda f_=emit_m1: f_(0))
                        pending.append(lambda f_=emit_m1: f_(1))

                # block loop. Score emission map: blocks 0-2 prefetched in
                # the proj phase; block 3 emitted through block 0's slots;
                # blocks 4-7 just-in-time (JIT_LEAD slots ahead in their own
                # loop, first JIT_LEAD groups at the previous block's tail).
                with (
                    tc.tile_pool(name="ps_y0", bufs=2, space="PSUM") as ps_y0pool,
                    tc.tile_pool(name="ps_y1", bufs=1, space="PSUM") as ps_y1pool,
                    tc.tile_pool(name="ps_o", bufs=1, space="PSUM") as ps_opool,
                ):
                    for bi, (c, p_) in enumerate(blocks):
                        hA, hB = 2 * p_, 2 * p_ + 1
                        psY = [
                            ps_y0pool.tile([DH + 1, CH], F32, tag="psY0", name="psY0"),
                            ps_y1pool.tile([DH + 1, CH], F32, tag="psY1", name="psY1"),
                        ]

                        def y_hB(kt, psY=psY, bi=bi, hB=hB):
                            pt = PT.pop((bi, kt))
                            nc.tensor.matmul(
                                psY[1][:],
                                V[kt][:, hB, :],
                                pt[:, CH : 2 * CH],
                                start=(kt == 0),
                                stop=(kt == nkt - 1),
                            )

                        for kt in range(nkt):
                            nc.tensor.matmul(
                                psY[0][:],
                                V[kt][:, hA, :],
                                PT[(bi, kt)][:, 0:CH],
                                start=(kt == 0),
                                stop=(kt == nkt - 1),
                            )
                            if kt >= 2:
                                y_hB(kt - 2)
                            # score emission for this slot
                            if bi == 0 and kt >= 2:
                                emit_score_exp(3, kt - 2)
                            elif bi >= 3 and kt >= nkt - JIT_LEAD and bi + 1 < len(blocks):
                                emit_score_exp(bi + 1, kt - (nkt - JIT_LEAD))
                            if bi >= 4 and kt < nkt - JIT_LEAD:
                                emit_score_exp(bi, kt + JIT_LEAD)
                            if pending:
                                pending.pop(0)()
                        for kt in (nkt - 2, nkt - 1):
                            y_hB(kt)
                            if bi == 0:
                                emit_score_exp(3, kt)
                        normalize(p_, c, psY, 0)
                        normalize(p_, c, psY, 1)
                        if p_ == 0:
                            queue_outproj_m0(c)
                        else:
                            queue_outproj_m1(c, on_psy0=(c == nch - 1))
                    while pending:
                        pending.pop(0)()

    nc.compile()
    return nc


def make_in_maps(inputs_q, inputs_kv, wq, wk, wv, wo):
    """Shard full inputs into 8 per-core input dicts (host-side).

    All tensors are pre-tiled so each device DMA piece is contiguous per
    SBUF partition (f = ft*128 + p on partition p)."""
    in_maps = []
    scale = 1.0 / np.sqrt(DH)
    nf = F // P

    def x_pre(x):  # [S, F] -> [P, nch, nf, CH]
        return np.ascontiguousarray(
            x.T.reshape(nf, P, S // CH, CH).transpose(1, 2, 0, 3)
        ).astype(np.float16)

    for core in range(NCORES):
        b = core // (NCORES // B)
        hg = core % (NCORES // B)
        hs = slice(hg * HPC, (hg + 1) * HPC)
        wq_c = (wq[:, hs, :] * scale).reshape(F, HPC * DH)
        wk_c = wk[:, hs, :].reshape(F, HPC * DH)
        wv_c = wv[:, hs, :].reshape(F, HPC * DH)
        wo_c = wo[hs].reshape(HPC * DH, O)
        in_maps.append(
            {
                "xq": x_pre(np.asarray(inputs_q[b])),
                "xkv": x_pre(np.asarray(inputs_kv[b])),
                # [F, hd] -> [P, npair, nf, 128]
                "wq": np.ascontiguousarray(
                    np.asarray(wq_c).reshape(nf, P, 2, P).transpose(1, 2, 0, 3)
                ).astype(np.float16),
                "wk": np.ascontiguousarray(
                    np.asarray(wk_c).reshape(nf, P, 2, P).transpose(1, 2, 0, 3)
                ).astype(np.float16),
                # [F, hd] -> [P, nf, hd]
                "wv": np.ascontiguousarray(
                    np.asarray(wv_c).reshape(nf, P, HPC * DH).transpose(1, 0, 2)
                ).astype(np.float16),
                # [hd, O] -> [P, hd//P, O]
                "wo": np.ascontiguousarray(
                    np.asarray(wo_c).reshape(2, P, O).transpose(1, 0, 2)
                ).astype(np.float16),
            }
        )
    return in_maps


_CACHE = {}


def _get_program():
    if "nc" not in _CACHE:
        _CACHE["nc"] = build_program()
    return _CACHE["nc"]


def run_sharded(inputs_q, inputs_kv, wq, wk, wv, wo, bo, **spmd_kwargs):
    """Build in_maps, run on 8 cores, reduce partials. Returns (out, results)."""
    nc = _get_program()
    in_maps = make_in_maps(inputs_q, inputs_kv, wq, wk, wv, wo)
    res = run_bass_kernel_spmd(nc, in_maps, core_ids=list(range(NCORES)), **spmd_kwargs)
    gpb = NCORES // B  # head-group cores per batch element
    out = np.zeros((B, S, O), dtype=np.float32)
    for core in range(NCORES):
        out[core // gpb] += res.results[core]["out"].astype(np.float32)
    out += np.asarray(bo, dtype=np.float32)
    return out, res


def kernel(inputs_q, inputs_kv, wq, wk, wv, wo, bo):
    out, _ = run_sharded(
        np.asarray(inputs_q),
        np.asarray(inputs_kv),
        np.asarray(wq),
        np.asarray(wk),
        np.asarray(wv),
        np.asarray(wo),
        np.asarray(bo),
    )
    return out


# revision 34
# speedup vs baseline: 1.0344x; 1.0344x over previous
"""Multi-head dot-product attention (B=2, S=2048, F=1024, H=16, DH=64, O=1024)
as a Bass/Tile kernel on 8 Trainium2 NeuronCores.

Sharding: data-parallel over B (2) x tensor-parallel over H (4 groups of 4
heads) = 8 cores. Each core computes q/k/v projections for its 4 heads,
softmax attention, and a partial output projection; the host sums the 4
partial outputs per batch element and adds the bias.

Device layouts (per core) are pre-tiled on the host so every DMA piece is
contiguous per SBUF partition (big descriptors; an f-strided source would
shatter into 1KB descriptors and cap each transfer at ~24GB/s):
  xq, xkv  [P, nch, nf, CH] fp16   x[c, ft, q] per partition
  wq, wk   [P, 2, nf, 128]  fp16   per-pair-major (wq pre-scaled 1/sqrt(DH))
  wv       [P, nf, hd]      fp16
  wo       [P, hd//P, O]    fp16
  out      [S, O]           fp16   partial output (host accumulates in f32)

Attention works in transposed-score space: sT[k, q] = KT_slice.T @ QT (two
heads packed into PE row-groups 0-63 / 64-127 run concurrently), one exp on
ACT covers both heads, then y'T = V'.T @ PT where V' carries a ones column
so row 64 of y'T accumulates the softmax denominator (scores are O(1), so
max-subtraction is unnecessary). The denominator row is partition-broadcast
on GPSIMD, reciprocated on DVE, and multiplied into fp16 yT tiles used as
lhsT of the output projection.

Scheduling, driven by two facts: (a) the ACT exp stream (~137us busy) is
the critical resource, (b) a score matmul for exp #n cannot start until
exp #n-2 frees its PSUM slot, so any PE work emitted after it inherits an
ACT-locked stall. Hence: blocks 0-2's score/exp groups dribble out during
the projection chains (filling ACT from ~14us), block 3's are emitted
through block 0's slots, and blocks 4-7 emit their own scores just-in-time
four k-slots ahead inside their own loops (first four at the previous
block's tail) so almost no PE work queues behind an ACT-locked matmul. The
hB y-stream is staggered two k-slots behind hA so its single-buffered PSUM
bank clears the previous block's inline normalization. The output
projection is split per pair: pair-0 partials (one matmul + fp16 cast)
drain during the pair-1 block, pair-1 add-units during the next block, so
only the last chunk's 8 add-units, the final normalization, and 4 stores
trail the last exp. Dummy warmup matmuls fill the chunk-0 DMA window to
hold the HAM activity up so projection chains run at 2.4GHz.
"""

import numpy as np

import concourse.bass as bass
import concourse.mybir as mybir
import concourse.tile as tile
from concourse import bacc
from concourse.bass_utils import run_bass_kernel_spmd

F32 = mybir.dt.float32
F16 = mybir.dt.float16
AF = mybir.ActivationFunctionType

B, S, F, H, DH, O = 2, 2048, 1024, 16, 64, 1024
NCORES = 8
HPC = 4  # heads per core
CH = 512  # q-chunk width
P = 128
NPAR = 3  # pt parity depth (blocks of exp tiles in flight)
JIT_LEAD = 4  # in-block score emission lead (k-slots)


def build_program(s=S, f=F, o=O, hpc=HPC):
    npair = hpc // 2
    nch = s // CH  # q chunks
    nkt = s // P  # k tiles
    nf = f // P  # contraction tiles for projections
    hd = hpc * DH  # stacked head dims per core (256)

    nc = bacc.Bacc("TRN2", target_bir_lowering=False, debug=False, num_devices=NCORES)

    xq_d = nc.dram_tensor("xq", [P, nch, nf, CH], F16, kind="ExternalInput")
    xkv_d = nc.dram_tensor("xkv", [P, nch, nf, CH], F16, kind="ExternalInput")
    wq_d = nc.dram_tensor("wq", [P, npair, nf, P], F16, kind="ExternalInput")
    wk_d = nc.dram_tensor("wk", [P, npair, nf, P], F16, kind="ExternalInput")
    wv_d = nc.dram_tensor("wv", [P, nf, hd], F16, kind="ExternalInput")
    wo_d = nc.dram_tensor("wo", [P, hd // P, o], F16, kind="ExternalInput")
    out = nc.dram_tensor("out", [s, o], F16, kind="ExternalOutput")

    with tile.TileContext(nc) as tc:
        with (
            tc.tile_pool(name="weights", bufs=1) as wpool,
            tc.tile_pool(name="xin", bufs=2) as xpool,
            tc.tile_pool(name="qkv", bufs=1) as qkvpool,
            tc.tile_pool(name="pt", bufs=1) as ptpool,
            tc.tile_pool(name="norm", bufs=2) as npool,
            tc.tile_pool(name="acc", bufs=1) as apool,
            tc.tile_pool(name="outsb", bufs=2) as opool,
        ):
            # ---- weights + constants -------------------------------------
            wq_sb = [wpool.tile([P, nf, P], F16, tag=f"wq{m}", name=f"wq{m}") for m in range(npair)]
            wk_sb = [wpool.tile([P, nf, P], F16, tag=f"wk{m}", name=f"wk{m}") for m in range(npair)]
            wv_sb = wpool.tile([P, nf, hd], F16, tag="wv")
            wo_sb = wpool.tile([P, hd // P, o], F16, tag="wo")
            # memset can't write fp16; memset fp32 scratch, cast-copy
            ones_f32 = wpool.tile([P, 4 * P], F32, tag="ones_f32")
            nc.vector.memset(ones_f32[:], 1.0)
            dummy16 = wpool.tile([1, 4 * P], F16, tag="dummy16")
            nc.vector.tensor_copy(dummy16[:], ones_f32[0:1, :])

            # ---- storage -------------------------------------------------
            QT = [
                [qkvpool.tile([P, CH], F16, tag=f"QT{p_}_{c}", name=f"QT{p_}_{c}") for c in range(nch)]
                for p_ in range(npair)
            ]
            KT = [
                [qkvpool.tile([P, CH], F16, tag=f"KT{p_}_{c}", name=f"KT{p_}_{c}") for c in range(nch)]
                for p_ in range(npair)
            ]
            # V': per k-tile [128, hpc, DH+1]; last column is ones
            V = [qkvpool.tile([P, hpc, DH + 1], F16, tag=f"V{kt}", name=f"V{kt}") for kt in range(nkt)]
            YT = [
                [qkvpool.tile([P, CH], F16, tag=f"YT{p_}_{c}", name=f"YT{p_}_{c}") for c in range(nch)]
                for p_ in range(npair)
            ]

            # ps_s (scores PSUM) lives for the whole kernel: 4 banks.
            # Projection-phase pools add 4 more (within the 8-bank budget);
            # after they close, psY0(2)+psY1(1)+ps_o(1) take the other 4.
            with tc.tile_pool(name="ps_att", bufs=2, space="PSUM") as ps_att:
                blocks = [(c, p_) for c in range(nch) for p_ in range(npair)]

                def emit_scores(p_, c, kt):
                    ps_s = ps_att.tile([P, 2 * CH], F32, tag="ps_s", name="ps_s")
                    nc.tensor.matmul(
                        ps_s[:, 0:CH],
                        KT[p_][kt // 4][0:DH, (kt % 4) * P : (kt % 4 + 1) * P],
                        QT[p_][c][0:DH, :],
                        tile_position=(0, 0),
                    )
                    nc.tensor.matmul(
                        ps_s[:, CH : 2 * CH],
                        KT[p_][kt // 4][DH : 2 * DH, (kt % 4) * P : (kt % 4 + 1) * P],
                        QT[p_][c][DH : 2 * DH, :],
                        tile_position=(DH, 0),
                    )
                    return ps_s

                # saved exp(scores) tiles; (block % NPAR, kt) keys the slot
                PT = {}

                def emit_score_exp(bi, kt):
                    c, p_ = blocks[bi]
                    ps_s = emit_scores(p_, c, kt)
                    pt = ptpool.tile(
                        [P, 2 * CH], F16, tag=f"pt{bi % NPAR}_{kt}", name=f"pt{bi % NPAR}_{kt}"
                    )
                    nc.scalar.activation(pt[:], ps_s[:], AF.Exp)
                    PT[(bi, kt)] = pt

                # prefetch bookkeeping for blocks 0..NPAR-1 (proj phase)
                pf_done = [set() for _ in range(NPAR)]
                qt_ready = [[False] * nch for _ in range(npair)]
                kt_ready = [[False] * nch for _ in range(npair)]

                def pf_drain(limit):
                    n = 0
                    for bi in range(min(NPAR, len(blocks))):
                        cb, pb = blocks[bi]
                        if not qt_ready[pb][cb]:
                            continue
                        for kt in range(nkt):
                            if n >= limit:
                                return
                            if kt in pf_done[bi] or not kt_ready[pb][kt // 4]:
                                continue
                            emit_score_exp(bi, kt)
                            pf_done[bi].add(kt)
                            n += 1

                # ---- projections (+ blocks 0-2 scores/exp woven in) ------
                with (
                    tc.tile_pool(name="ps_projqk", bufs=1, space="PSUM") as ps_projqk,
                    tc.tile_pool(name="ps_projv", bufs=2, space="PSUM") as ps_projv,
                ):
                    # PE warm-up: the HAM raises the PE clock only after
                    # ~10us of SUSTAINED activity, so fill the chunk-0 DMA
                    # wait with back-to-back dummy matmuls
                    for wu in range(4):
                        ps_wu = ps_projv.tile([P, CH], F32, tag="psV", name="ps_wu")
                        nc.tensor.matmul(ps_wu[:], dummy16[0:1, 0:P], dummy16[0:1, :])
                    for kt in range(nkt):
                        nc.vector.tensor_copy(V[kt][:, :, DH], ones_f32[:, 0:hpc])

                    XQP = [(0, 2), (2, 4), (4, 8)]  # xq piece ft ranges
                    XKP = [(0, 2), (2, 5), (5, 8)]  # xkv piece ft ranges
                    for c in range(nch):
                        cs = slice(c * CH, (c + 1) * CH)
                        xq_p = [
                            xpool.tile([P, b - a, CH], F16, tag=f"xq{i}", name=f"xq{i}")
                            for i, (a, b) in enumerate(XQP)
                        ]
                        xkv_p = [
                            xpool.tile([P, b - a, CH], F16, tag=f"xkv{i}", name=f"xkv{i}")
                            for i, (a, b) in enumerate(XKP)
                        ]

                        def xq_ap(ft):
                            for i, (a, b) in enumerate(XQP):
                                if a <= ft < b:
                                    return xq_p[i][:, ft - a]
                            raise AssertionError

                        def xkv_ap(ft):
                            for i, (a, b) in enumerate(XKP):
                                if a <= ft < b:
                                    return xkv_p[i][:, ft - a]
                            raise AssertionError

                        if c == 0:
                            # critical-path transfers on both rings, in
                            # consumption order
                            nc.sync.dma_start(wq_sb[0][:], wq_d.ap()[:, 0])
                            for i, (a, b) in enumerate(XQP):
                                nc.sync.dma_start(xq_p[i][:], xq_d.ap()[:, c, a:b, :])
                            nc.sync.dma_start(wq_sb[1][:], wq_d.ap()[:, 1])
                            nc.scalar.dma_start(wk_sb[0][:], wk_d.ap()[:, 0])
                            for i, (a, b) in enumerate(XKP):
                                nc.scalar.dma_start(xkv_p[i][:], xkv_d.ap()[:, c, a:b, :])
                            nc.scalar.dma_start(wk_sb[1][:], wk_d.ap()[:, 1])
                            nc.scalar.dma_start(wv_sb[:], wv_d.ap())
                        else:
                            if c == 1:
                                nc.sync.dma_start(wo_sb[:], wo_d.ap())
                            for i, (a, b) in enumerate(XQP):
                                nc.sync.dma_start(xq_p[i][:], xq_d.ap()[:, c, a:b, :])
                            for i, (a, b) in enumerate(XKP):
                                nc.sync.dma_start(xkv_p[i][:], xkv_d.ap()[:, c, a:b, :])

                        # per-pair projection chains; K copies land right
                        # after each pair's Q chain so its score groups can
                        # start the prefetch early
                        for m in range(npair):
                            psQ = ps_projqk.tile([P, CH], F32, tag=f"psQK{m}", name="psQ")
                            for ft in range(nf):
                                nc.tensor.matmul(
                                    psQ[:],
                                    wq_sb[m][:, ft],
                                    xq_ap(ft),
                                    start=(ft == 0),
                                    stop=(ft == nf - 1),
                                )
                            nc.vector.tensor_copy(QT[m][c][:], psQ[:])
                            qt_ready[m][c] = True
                            pf_drain(2)
                            psK = ps_projqk.tile([P, CH], F32, tag=f"psQK{m}", name="psK")
                            for ft in range(nf):
                                nc.tensor.matmul(
                                    psK[:],
                                    wk_sb[m][:, ft],
                                    xkv_ap(ft),
                                    start=(ft == 0),
                                    stop=(ft == nf - 1),
                                )
                            nc.vector.tensor_copy(KT[m][c][:], psK[:])
                            kt_ready[m][c] = True
                            pf_drain(2)
                        # V pass (xkv chunk tile as lhsT); one PSUM
                        # accumulation group per bank, so st is outer
                        for st in range(4):
                            psV = ps_projv.tile([P, CH], F32, tag="psV", name="psV")
                            for ft in range(nf):
                                nc.tensor.matmul(
                                    psV[:, 0:hd],
                                    xkv_ap(ft)[:, st * P : (st + 1) * P],
                                    wv_sb[:, ft, :],
                                    start=(ft == 0),
                                    stop=(ft == nf - 1),
                                )
                            kt = c * 4 + st
                            nc.vector.tensor_copy(
                                V[kt][:, :, 0:DH],
                                psV[:, 0:hd].rearrange("p (h d) -> p h d", h=hpc),
                            )
                            pf_drain(3)
                    pf_drain(99)

                # deferred output-projection queue: sub-microsecond PE units
                # injected into later k-slots so the PE never bursts
                pending = []

                def normalize(p_, c, psY, h01):
                    den_r = npool.tile([1, CH], F32, tag="den", name="den_r")
                    nc.vector.tensor_copy(den_r[:], psY[h01][DH : DH + 1, :])
                    bc_sb = npool.tile([DH, CH], F32, tag="bc", name="bc_sb")
                    nc.gpsimd.partition_broadcast(bc_sb[:], den_r[:])
                    inv_sb = npool.tile([DH, CH], F32, tag="inv", name="inv_sb")
                    nc.vector.reciprocal_approx_fast(out=inv_sb[:], in_=bc_sb[:])
                    nc.vector.tensor_tensor(
                        YT[p_][c][h01 * DH : (h01 + 1) * DH, :],
                        psY[h01][0:DH, :],
                        inv_sb[:],
                        mybir.AluOpType.mult,
                    )

                acc32 = {}

                def queue_outproj(c):
                    # full (st, j) units: two accumulating matmuls + fp16
                    # cast, store dispatched with the second half
                    for st in range(4):
                        qt = c * 4 + st
                        carrier = {}

                        def emit_half(j, st=st, c=c, qt=qt, carrier=carrier):
                            if j == 0:
                                carrier["out16"] = opool.tile([P, o], F16, tag="out16", name="out16")
                            ps_o = ps_opool.tile([P, CH], F32, tag="ps_o", name="ps_o")
                            for m in range(hd // P):
                                nc.tensor.matmul(
                                    ps_o[:],
                                    YT[m][c][:, st * P : (st + 1) * P],
                                    wo_sb[:, m, j * CH : (j + 1) * CH],
                                    start=(m == 0),
                                    stop=(m == hd // P - 1),
                                )
                            nc.vector.tensor_copy(
                                carrier["out16"][:, j * CH : (j + 1) * CH], ps_o[:]
                            )
                            if j == 1:
                                nc.sync.dma_start(
                                    out.ap()[qt * P : (qt + 1) * P, :], carrier["out16"][:]
                                )

                        pending.append(lambda f_=emit_half: f_(0))
                        pending.append(lambda f_=emit_half: f_(1))

                def queue_outproj_m0(c):
                    # pair-0 partials into SBUF fp16 while pair 1 is still
                    # accumulating its y
                    for st in range(4):
                        for j in range(2):

                            def emit_m0(st=st, j=j, c=c):
                                ps_o = ps_opool.tile([P, CH], F32, tag="ps_o", name="ps_o")
                                nc.tensor.matmul(
                                    ps_o[:],
                                    YT[0][c][:, st * P : (st + 1) * P],
                                    wo_sb[:, 0, j * CH : (j + 1) * CH],
                                )
                                a32 = apool.tile(
                                    [P, CH], F16, tag=f"a32_{st}_{j}", name="a32"
                                )
                                nc.vector.tensor_copy(a32[:], ps_o[:])
                                acc32[(st, j)] = a32

                            pending.append(emit_m0)

                def queue_outproj_m1(c, on_psy0=False):
                    # one matmul + one fused add-cast per (st, j); the last
                    # chunk's run on the freed psY0 slots (bufs=2) so the
                    # tail chain pipelines
                    for st in range(4):
                        qt = c * 4 + st
                        carrier = {}

                        def emit_m1(j, st=st, c=c, qt=qt, carrier=carrier):
                            if j == 0:
                                carrier["out16"] = opool.tile([P, o], F16, tag="out16", name="out16")
                            if on_psy0:
                                ps_o = ps_y0pool.tile([P, CH], F32, tag="psY0", name="ps_o1")
                            else:
                                ps_o = ps_opool.tile([P, CH], F32, tag="ps_o", name="ps_o1")
                            nc.tensor.matmul(
                                ps_o[:],
                                YT[1][c][:, st * P : (st + 1) * P],
                                wo_sb[:, 1, j * CH : (j + 1) * CH],
                            )
                            nc.vector.tensor_tensor(
                                carrier["out16"][:, j * CH : (j + 1) * CH],
                                acc32.pop((st, j))[:],
                                ps_o[:],
                                mybir.AluOpType.add,
                            )
                            if j == 1:
                                nc.sync.dma_start(
                                    out.ap()[qt * P : (qt + 1) * P, :], carrier["out16"][:]
                                )

                        pending.append(lambda f_=emit_m1: f_(0))
                        pending.append(lambda f_=emit_m1: f_(1))

                # block loop. Score emission map: blocks 0-2 prefetched in
                # the proj phase; block 3 emitted through block 0's slots;
                # blocks 4-7 just-in-time (JIT_LEAD slots ahead in their own
                # loop, first JIT_LEAD groups at the previous block's tail).
                with (
                    tc.tile_pool(name="ps_y0", bufs=2, space="PSUM") as ps_y0pool,
                    tc.tile_pool(name="ps_y1", bufs=1, space="PSUM") as ps_y1pool,
                    tc.tile_pool(name="ps_o", bufs=1, space="PSUM") as ps_opool,
                ):
                    for bi, (c, p_) in enumerate(blocks):
                        hA, hB = 2 * p_, 2 * p_ + 1
                        last2 = bi >= 4
                        psY = [
                            ps_y0pool.tile([DH + 1, CH], F32, tag="psY0", name="psY0"),
                            ps_y1pool.tile([DH + 1, CH], F32, tag="psY1", name="psY1"),
                        ]

                        def y_hB(kt, psY=psY, bi=bi, hB=hB):
                            pt = PT.pop((bi, kt))
                            nc.tensor.matmul(
                                psY[1][:],
                                V[kt][:, hB, :],
                                pt[:, CH : 2 * CH],
                                start=(kt == 0),
                                stop=(kt == nkt - 1),
                            )

                        for kt in range(nkt):
                            nc.tensor.matmul(
                                psY[0][:],
                                V[kt][:, hA, :],
                                PT[(bi, kt)][:, 0:CH],
                                start=(kt == 0),
                                stop=(kt == nkt - 1),
                            )
                            if kt >= 2:
                                y_hB(kt - 2)
                                if bi + NPAR < len(blocks):
                                    emit_score_exp(bi + NPAR, kt - 2)
                            if pending:
                                pending.pop(0)()
                            if last2 and pending:
                                pending.pop(0)()
                        for kt in (nkt - 2, nkt - 1):
                            y_hB(kt)
                            if bi + NPAR < len(blocks):
                                emit_score_exp(bi + NPAR, kt)
                        if bi == len(blocks) - 1:
                            # dependency-free fillers hold the PE p-state
                            # through the final normalization latency
                            for _ in range(10):
                                ps_fill = ps_opool.tile([P, CH], F32, tag="ps_o", name="ps_fill")
                                nc.tensor.matmul(ps_fill[:, 0:P], dummy16[0:1, 0:P], dummy16[0:1, 0:P])
                        normalize(p_, c, psY, 0)
                        normalize(p_, c, psY, 1)
                        if p_ == npair - 1:
                            if c == nch - 1:
                                queue_outproj_m1(c, on_psy0=True)
                            else:
                                queue_outproj(c)
                        elif c == nch - 1:
                            queue_outproj_m0(c)
                    while pending:
                        pending.pop(0)()

    nc.compile()
    return nc


def make_in_maps(inputs_q, inputs_kv, wq, wk, wv, wo):
    """Shard full inputs into 8 per-core input dicts (host-side).

    All tensors are pre-tiled so each device DMA piece is contiguous per
    SBUF partition (f = ft*128 + p on partition p)."""
    in_maps = []
    scale = 1.0 / np.sqrt(DH)
    nf = F // P

    def x_pre(x):  # [S, F] -> [P, nch, nf, CH]
        return np.ascontiguousarray(
            x.T.reshape(nf, P, S // CH, CH).transpose(1, 2, 0, 3)
        ).astype(np.float16)

    for core in range(NCORES):
        b = core // (NCORES // B)
        hg = core % (NCORES // B)
        hs = slice(hg * HPC, (hg + 1) * HPC)
        wq_c = (wq[:, hs, :] * scale).reshape(F, HPC * DH)
        wk_c = wk[:, hs, :].reshape(F, HPC * DH)
        wv_c = wv[:, hs, :].reshape(F, HPC * DH)
        wo_c = wo[hs].reshape(HPC * DH, O)
        in_maps.append(
            {
                "xq": x_pre(np.asarray(inputs_q[b])),
                "xkv": x_pre(np.asarray(inputs_kv[b])),
                # [F, hd] -> [P, npair, nf, 128]
                "wq": np.ascontiguousarray(
                    np.asarray(wq_c).reshape(nf, P, 2, P).transpose(1, 2, 0, 3)
                ).astype(np.float16),
                "wk": np.ascontiguousarray(
                    np.asarray(wk_c).reshape(nf, P, 2, P).transpose(1, 2, 0, 3)
                ).astype(np.float16),
                # [F, hd] -> [P, nf, hd]
                "wv": np.ascontiguousarray(
                    np.asarray(wv_c).reshape(nf, P, HPC * DH).transpose(1, 0, 2)
                ).astype(np.float16),
                # [hd, O] -> [P, hd//P, O]
                "wo": np.ascontiguousarray(
                    np.asarray(wo_c).reshape(2, P, O).transpose(1, 0, 2)
                ).astype(np.float16),
            }
        )
    return in_maps


_CACHE = {}


def _get_program():
    if "nc" not in _CACHE:
        _CACHE["nc"] = build_program()
    return _CACHE["nc"]


def run_sharded(inputs_q, inputs_kv, wq, wk, wv, wo, bo, **spmd_kwargs):
    """Build in_maps, run on 8 cores, reduce partials. Returns (out, results)."""
    nc = _get_program()
    in_maps = make_in_maps(inputs_q, inputs_kv, wq, wk, wv, wo)
    res = run_bass_kernel_spmd(nc, in_maps, core_ids=list(range(NCORES)), **spmd_kwargs)
    gpb = NCORES // B  # head-group cores per batch element
    out = np.zeros((B, S, O), dtype=np.float32)
    for core in range(NCORES):
        out[core // gpb] += res.results[core]["out"].astype(np.float32)
    out += np.asarray(bo, dtype=np.float32)
    return out, res


def kernel(inputs_q, inputs_kv, wq, wk, wv, wo, bo):
    out, _ = run_sharded(
        np.asarray(inputs_q),
        np.asarray(inputs_kv),
        np.asarray(wq),
        np.asarray(wk),
        np.asarray(wv),
        np.asarray(wo),
        np.asarray(bo),
    )
    return out
